# revision 20
# baseline (speedup 1.0000x reference)
"""BiLSTM tagger kernel, direction-parallel sharding over 8 NeuronCores.

Core c in 0..3 runs the FORWARD direction for sequences [32c, 32c+32);
core c+4 runs the BACKWARD direction for the same sequences (inputs
pre-reversed per sequence length on the host). Every matmul therefore has
M=32 batch rows instead of 16 — recurrent matmul cost is N-bound, so this
halves per-core PE work per step — and each core runs ONE scan per layer,
halving ScalarE/VectorE contention.

Between layers, the forward/backward halves are exchanged with a pairwise
AllGather (replica groups {c, c+4}) of the layer's scan-order output into
hpair [2*ntok, H]: slot 0 = lower rank = forward data on BOTH cores, so
the program is identical across cores; all per-core asymmetry (reversal
indices, which weights, which token half of the classifier) lives in
host-computed input tensors. Gate order i,f,o,g (sigmoid block first),
all matmuls bf16, PSUM fp32, gx injected into PSUM via identity matmuls
one step ahead.
"""

import sys

for _p in ("/opt/trn_rl_repo",):
    if _p not in sys.path:
        sys.path.append(_p)

import numpy as np
import ml_dtypes

import concourse.bass as bass
import concourse.tile as tile
from concourse import bacc, mybir
from concourse.bass import IndirectOffsetOnAxis
from concourse.bass_utils import run_bass_kernel_spmd

F32 = mybir.dt.float32
BF16 = mybir.dt.bfloat16
I32 = mybir.dt.int32
AF = mybir.ActivationFunctionType
ALU = mybir.AluOpType

B, T, V, E, H, TAGS = 128, 512, 50000, 256, 512, 64
NC = 8
NPAIR = NC // 2          # 4 sequence groups
BL = B // NPAIR          # 32 sequences per core (one direction each)
G = 4 * H
GROUPS = [[c, c + NPAIR] for c in range(NPAIR)]

# gate order i,g,f,o: half A = (i,g) finishes first so t2 = sig(i)*tanh(g)
# overlaps half B's matmuls; half B = (f,o) needs only ONE sigmoid call
_GATE_PERM = np.concatenate([
    np.arange(0, H), np.arange(2 * H, 3 * H), np.arange(H, 2 * H),
    np.arange(3 * H, 4 * H)])


def _build(nc, Tn=T, Bl=BL, TC=2, RC=4):
    ntok = Bl * Tn            # 16384 per core
    nchunk = ntok // 128      # 128
    ncls = ntok // 2 // 128   # 64 classifier chunks (half the pair's tokens)
    KE = E // 128
    KH2 = 2 * H // 128
    KH = H // 128

    # ---- dram I/O (per-core data resolves fwd/bwd asymmetry) ----
    emb = nc.dram_tensor("emb", [V, E], F32, kind="ExternalInput")
    xg_idx = nc.dram_tensor("xg_idx", [128, nchunk], I32, kind="ExternalInput")
    iA2 = nc.dram_tensor("iA2", [128, nchunk], I32, kind="ExternalInput")
    iB2 = nc.dram_tensor("iB2", [128, nchunk], I32, kind="ExternalInput")
    icA = nc.dram_tensor("icA", [128, ncls], I32, kind="ExternalInput")
    icB = nc.dram_tensor("icB", [128, ncls], I32, kind="ExternalInput")
    mask = nc.dram_tensor("mask", [Bl, Tn], F32, kind="ExternalInput")
    ident = nc.dram_tensor("ident", [32, 32], BF16, kind="ExternalInput")

    wih, whh, biasd = {}, {}, {}
    for l, din in (("l1", E), ("l2", 2 * H)):
        wih[l] = nc.dram_tensor(f"wihT_{l}", [din, G], BF16, kind="ExternalInput")
        whh[l] = nc.dram_tensor(f"whhT_{l}", [H, G], BF16, kind="ExternalInput")
        biasd[l] = nc.dram_tensor(f"bias_{l}", [128, G], F32, kind="ExternalInput")
    wcls = nc.dram_tensor("wclsT", [2 * H, TAGS], BF16, kind="ExternalInput")
    bcls = nc.dram_tensor("bcls", [TAGS, 1], F32, kind="ExternalInput")

    gx = {l: nc.dram_tensor(f"gx_{l}", [ntok, G], BF16) for l in ("l1", "l2")}
    hloc = {l: nc.dram_tensor(f"hloc_{l}", [ntok, H], BF16) for l in ("l1", "l2")}
    hpair = {l: nc.dram_tensor(f"hpair_{l}", [2 * ntok, H], BF16)
             for l in ("l1", "l2")}
    logitsT = nc.dram_tensor("logitsT", [TAGS, ntok // 2], F32,
                             kind="ExternalOutput")

    with tile.TileContext(nc) as tc:
        with tc.tile_pool(name="const", bufs=1) as cpool:
            def load_const(nm, shape, dt, src_ap):
                t = cpool.tile(shape, dt, name=nm, tag=nm)
                nc.gpsimd.dma_start(t[:], src_ap)
                return t

            xg_sb = load_const("xg_sb", [128, nchunk], I32, xg_idx[:])
            iA2_sb = load_const("iA2_sb", [128, nchunk], I32, iA2[:])
            iB2_sb = load_const("iB2_sb", [128, nchunk], I32, iB2[:])
            icA_sb = load_const("icA_sb", [128, ncls], I32, icA[:])
            icB_sb = load_const("icB_sb", [128, ncls], I32, icB[:])
            mask_sb = load_const("mask_sb", [Bl, Tn], F32, mask[:])
            id_sb = load_const("id_sb", [32, 32], BF16, ident[:])
            bcls_sb = load_const("bcls_sb", [TAGS, 1], F32, bcls[:])
            bias_sb = {l: load_const(f"bias_sb_{l}", [128, G], F32, biasd[l][:])
                       for l in ("l1", "l2")}
            wcls_sb = cpool.tile([128, KH2, TAGS], BF16, name="wcls_sb")
            for k in range(KH2):
                nc.gpsimd.dma_start(wcls_sb[:, k, :], wcls[128 * k:128 * (k + 1), :])

            # layer-1 proj (emb gather) + scan + exchange
            _proj(nc, tc, nchunk, KE, wih["l1"], bias_sb["l1"], gx["l1"],
                  emb, xg_sb, None, None, is_emb=True)
            _scan(nc, tc, Tn, Bl, TC, RC, KH, whh["l1"], gx["l1"], hloc["l1"],
                  mask_sb, id_sb)
            nc.gpsimd.collective_compute(
                "AllGather", ALU.bypass, GROUPS,
                ins=[hloc["l1"][:]], outs=[hpair["l1"][:]])
            # layer-2 proj (gathers from hpair) + scan + exchange
            _proj(nc, tc, nchunk, KH2, wih["l2"], bias_sb["l2"], gx["l2"],
                  hpair["l1"], None, iA2_sb, iB2_sb, is_emb=False)
            _scan(nc, tc, Tn, Bl, TC, RC, KH, whh["l2"], gx["l2"], hloc["l2"],
                  mask_sb, id_sb)
            nc.gpsimd.collective_compute(
                "AllGather", ALU.bypass, GROUPS,
                ins=[hloc["l2"][:]], outs=[hpair["l2"][:]])

            # classifier over this core's half of the pair's tokens
            with tc.tile_pool(name="cls", bufs=3) as gp, \
                 tc.tile_pool(name="clsT", bufs=3) as gtp, \
                 tc.tile_pool(name="clsps", bufs=4, space="PSUM") as pp, \
                 tc.tile_pool(name="clso", bufs=3) as op:
                for c in range(ncls):
                    o2 = gp.tile([128, 2 * H], BF16, tag="in")
                    nc.gpsimd.indirect_dma_start(
                        out=o2[:, 0:H], out_offset=None, in_=hpair["l2"][:],
                        in_offset=IndirectOffsetOnAxis(ap=icA_sb[:, c:c + 1], axis=0))
                    nc.gpsimd.indirect_dma_start(
                        out=o2[:, H:2 * H], out_offset=None, in_=hpair["l2"][:],
                        in_offset=IndirectOffsetOnAxis(ap=icB_sb[:, c:c + 1], axis=0))
                    o2T = gtp.tile([128, KH2, 128], BF16, tag="inT")
                    for k in range(KH2):
                        nc.sync.dma_start_transpose(
                            o2T[:, k, :], o2[:, 128 * k:128 * (k + 1)])
                    ps = pp.tile([TAGS, 128], F32, name="clsps_t")
                    for k in range(KH2):
                        nc.tensor.matmul(ps[:], wcls_sb[:, k, :], o2T[:, k, :],
                                         start=(k == 0), stop=(k == KH2 - 1))
                    lg = op.tile([TAGS, 128], F32, tag="lg")
                    nc.scalar.activation(lg[:], ps[:], AF.Identity,
                                         bias=bcls_sb[:, 0:1])
                    nc.gpsimd.dma_start(logitsT[:, 128 * c:128 * (c + 1)], lg[:])
    return nc


def _proj(nc, tc, nchunk, KD, wih_d, bias_t, gx_d, src, emb_idx, iA, iB, is_emb):
    """gx = input @ W_ih^T + b in scan-time order. Layer 1: fp32 emb row
    gather + cast. Layer 2: two bf16 row gathers from hpair."""
    D = KD * 128
    G_ = G
    with tc.tile_pool(name="pw", bufs=1) as wpool, \
         tc.tile_pool(name="pg", bufs=3) as gpool, \
         tc.tile_pool(name="pgT", bufs=3) as tpool, \
         tc.tile_pool(name="pps", bufs=4, space="PSUM") as ppool, \
         tc.tile_pool(name="pout", bufs=3) as opool:
        wsb = wpool.tile([128, KD, G_], BF16, tag="w", name="wih_sb")
        for k in range(KD):
            nc.gpsimd.dma_start(wsb[:, k, :], wih_d[128 * k:128 * (k + 1), :])
        for c in range(nchunk):
            if is_emb:
                e32 = gpool.tile([128, D], F32, tag="e32")
                nc.gpsimd.indirect_dma_start(
                    out=e32[:], out_offset=None, in_=src[:],
                    in_offset=IndirectOffsetOnAxis(ap=emb_idx[:, c:c + 1], axis=0))
                xin = gpool.tile([128, D], BF16, tag="e16")
                nc.vector.tensor_copy(xin[:], e32[:])
            else:
                xin = gpool.tile([128, D], BF16, tag="e16")
                nc.gpsimd.indirect_dma_start(
                    out=xin[:, 0:H], out_offset=None, in_=src[:],
                    in_offset=IndirectOffsetOnAxis(ap=iA[:, c:c + 1], axis=0))
                nc.gpsimd.indirect_dma_start(
                    out=xin[:, H:2 * H], out_offset=None, in_=src[:],
                    in_offset=IndirectOffsetOnAxis(ap=iB[:, c:c + 1], axis=0))
            xT = tpool.tile([128, KD, 128], BF16, tag="xT")
            for k in range(KD):
                nc.sync.dma_start_transpose(xT[:, k, :], xin[:, 128 * k:128 * (k + 1)])
            gout = opool.tile([128, G_], BF16, tag="gout")
            for n in range(G_ // 512):
                ps = ppool.tile([128, 512], F32, name="pps")
                for k in range(KD):
                    nc.tensor.matmul(
                        ps[:], xT[:, k, :], wsb[:, k, 512 * n:512 * (n + 1)],
                        start=(k == 0), stop=(k == KD - 1))
                nc.vector.tensor_tensor(
                    out=gout[:, 512 * n:512 * (n + 1)], in0=ps[:],
                    in1=bias_t[:, 512 * n:512 * (n + 1)], op=ALU.add)
            nc.gpsimd.dma_start(gx_d[128 * c:128 * (c + 1), :], gout[:])


def _scan(nc, tc, Tn, Bl, TC, RC, KH, whh_d, gx_d, hout_d, mask_sb, id_sb):
    """Single-direction scan, M=32 batch. Same software-pipelined gx
    injection as the 2-scan variant, one scan per core."""
    gxv = gx_d.ap().rearrange("(b t) d -> b t d", b=Bl)
    houtv = hout_d.ap().rearrange("(b t) d -> b t d", b=Bl)
    H2 = 2 * H
    with tc.tile_pool(name="sw", bufs=1) as wpool, \
         tc.tile_pool(name="sgx", bufs=4) as gxpool, \
         tc.tile_pool(name="sst", bufs=1) as stpool, \
         tc.tile_pool(name="sps", bufs=4, space="PSUM") as pspool, \
         tc.tile_pool(name="swk", bufs=3) as wkpool, \
         tc.tile_pool(name="shT", bufs=3) as htpool, \
         tc.tile_pool(name="srng", bufs=3) as rpool:
        wsb = wpool.tile([128, KH, G], BF16, tag="whh", name="whh_sb")
        for k in range(KH):
            nc.gpsimd.dma_start(wsb[:, k, :], whh_d[128 * k:128 * (k + 1), :])
        hT = [htpool.tile([128, KH * Bl], BF16, tag="hT", name="hT0")]
        nc.vector.memset(hT[0][:], 0.0)
        c_st = stpool.tile([Bl, H], F32, tag="c", name="c_st")
        nc.vector.memset(c_st[:], 0.0)
        gxc = {}
        gA = [None]
        gB = [None]
        ring = [None]
        nwin = (Tn + TC - 1) // TC

        def load_gx(w):
            tl = gxpool.tile([Bl, TC, G], BF16, tag="gx", name="gxc")
            nc.gpsimd.dma_start(tl[:], gxv[:, w * TC:(w + 1) * TC, :])
            gxc[w] = tl
            gxc.pop(w - 3, None)

        def inject(tt):
            gA[0] = pspool.tile([Bl, H2], F32, tag="ps", name="gA")
            gB[0] = pspool.tile([Bl, H2], F32, tag="ps", name="gB")
            gxt = gxc[tt // TC]
            for half, lo in ((gA[0], 0), (gB[0], H2)):
                for n in range(2):
                    nc.tensor.matmul(
                        half[:, 512 * n:512 * (n + 1)], id_sb[:],
                        gxt[:, tt % TC, lo + 512 * n:lo + 512 * (n + 1)],
                        start=True, stop=False, skip_group_check=True)

        load_gx(0)
        if nwin > 1:
            load_gx(1)
        inject(0)
        for t in range(Tn):
            gAc, gBc = gA[0], gB[0]
            gact = wkpool.tile([Bl, G], F32, tag="gact", name="gact")
            t1 = wkpool.tile([Bl, H], F32, tag="t1", name="t1")
            t2 = wkpool.tile([Bl, H], F32, tag="t2", name="t2")
            if t % RC == 0:
                ring[0] = rpool.tile([Bl, RC, H], BF16, tag="ring", name="ring")
            for half, cols in ((gAc, (0, 1)), (gBc, (2, 3))):
                for n in cols:
                    dst_lo = 512 * (n % 2)
                    for k in range(KH):
                        nc.tensor.matmul(
                            half[:, dst_lo:dst_lo + 512],
                            hT[0][:, Bl * k:Bl * (k + 1)],
                            wsb[:, k, 512 * n:512 * (n + 1)],
                            start=False, stop=(k == KH - 1),
                            skip_group_check=True)
                if half is gAc:
                    # i,g activations + t2 run under half B's matmuls
                    nc.scalar.activation(gact[:, 0:H], gAc[:, 0:H], AF.Sigmoid)
                    nc.scalar.activation(gact[:, H:H2], gAc[:, H:H2], AF.Tanh)
                    nc.vector.tensor_tensor(out=t2[:], in0=gact[:, 0:H],
                                            in1=gact[:, H:H2], op=ALU.mult)
            # sig(f) depends only on half B's f-columns (n=2 chain), so it
            # fires ~0.9us before the o-columns finish; sig(o) is off the
            # c-path and runs while the DVE does t1/c
            nc.scalar.activation(gact[:, H2:3 * H], gBc[:, 0:H], AF.Sigmoid)
            nc.vector.tensor_tensor(out=t1[:], in0=gact[:, H2:3 * H],
                                    in1=c_st[:], op=ALU.mult)
            nc.vector.tensor_tensor(out=c_st[:], in0=t1[:], in1=t2[:], op=ALU.add)
            tch = wkpool.tile([Bl, H], F32, tag="tch", name="tch")
            nc.scalar.activation(tch[:], c_st[:], AF.Tanh)
            nc.scalar.activation(gact[:, 3 * H:G], gBc[:, H:H2], AF.Sigmoid)
            h16 = wkpool.tile([Bl, H], BF16, tag="h16", name="h16")
            nc.vector.tensor_tensor(out=h16[:], in0=gact[:, 3 * H:G],
                                    in1=tch[:], op=ALU.mult)
            hT_ps = pspool.tile([128, KH * Bl], F32, tag="ps", name="hT_ps")
            if t + 1 < Tn:
                if (t + 1) % TC == 0 and (t + 1) // TC + 1 < nwin:
                    load_gx((t + 1) // TC + 1)
                inject(t + 1)
            # per-chunk copy right behind each transpose so next step's
            # first matmul (which reads only chunk 0) starts early
            hTn = htpool.tile([128, KH * Bl], BF16, tag="hT", name="hTn")
            for k in range(KH):
                nc.tensor.matmul(hT_ps[:, Bl * k:Bl * (k + 1)],
                                 h16[:, 128 * k:128 * (k + 1)], id_sb[:],
                                 start=True, stop=True)
                nc.scalar.activation(hTn[:, Bl * k:Bl * (k + 1)],
                                     hT_ps[:, Bl * k:Bl * (k + 1)], AF.Copy)
            hT[0] = hTn
            nc.vector.tensor_scalar_mul(ring[0][:, t % RC, :], h16[:],
                                        mask_sb[:, t:t + 1])
            if (t + 1) % RC == 0:
                t0r = t + 1 - RC
                nc.gpsimd.dma_start(houtv[:, t0r:t0r + RC, :], ring[0][:, :, :])


def _prep_inputs(inputs, Tn=T, Bl=BL):
    x = np.asarray(inputs["x"]).astype(np.int32)
    lengths = np.asarray(inputs["lengths"]).astype(np.int32)
    emb = np.asarray(inputs["emb"], dtype=np.float32)
    ntok = Bl * Tn
    bf = ml_dtypes.bfloat16

    wt = {}
    for s in ("f1", "b1", "f2", "b2"):
        w_ih = np.asarray(inputs[f"W_ih_{s}"], np.float32)[_GATE_PERM]
        w_hh = np.asarray(inputs[f"W_hh_{s}"], np.float32)[_GATE_PERM]
        b = np.asarray(inputs[f"b_{s}"], np.float32)[_GATE_PERM]
        wt[f"wihT_{s}"] = np.ascontiguousarray(w_ih.T).astype(bf)
        wt[f"whhT_{s}"] = np.ascontiguousarray(w_hh.T).astype(bf)
        wt[f"bias_{s}"] = np.tile(b.reshape(1, G), (128, 1))
    com = {"emb": emb, "ident": np.eye(32, dtype=bf),
           "wclsT": np.ascontiguousarray(
               np.asarray(inputs["W_cls"], np.float32).T).astype(bf),
           "bcls": np.asarray(inputs["b_cls"], np.float32).reshape(TAGS, 1)}

    def chunked(a):
        return np.ascontiguousarray(a.reshape(-1).reshape(-1, 128).T)

    in_maps = [None] * NC
    for p in range(NPAIR):
        xs = x[Bl * p:Bl * (p + 1), :Tn]
        ls = np.minimum(lengths[Bl * p:Bl * (p + 1)], Tn)
        ts = np.arange(Tn)[None, :]
        rev = np.where(ts < ls[:, None], ls[:, None] - 1 - ts, ts)  # [Bl,Tn]
        base = np.arange(Bl)[:, None] * Tn + ts                      # natural
        base_rev = np.arange(Bl)[:, None] * Tn + rev                 # reversed
        m_common = {"mask": (ts < ls[:, None]).astype(np.float32)}
        m_common.update(com)

        # classifier token halves: fwd core -> seqs [0:Bl//2), bwd -> rest
        def cls_idx(b0, slotA_rev):
            tok = (np.arange(b0 * Tn, (b0 + Bl // 2) * Tn))
            bb, tt2 = tok // Tn, tok % Tn
            iA_ = bb * Tn + tt2
            iB_ = ntok + bb * Tn + rev[bb, tt2]
            if slotA_rev:
                pass
            return chunked(iA_.astype(np.int32)), chunked(iB_.astype(np.int32))

        for half, core in ((0, p), (1, p + NPAIR)):
            if half == 0:   # forward core
                m = {"xg_idx": chunked(xs),
                     "iA2": chunked(base.astype(np.int32)),
                     "iB2": chunked((ntok + base_rev).astype(np.int32)),
                     "wihT_l1": wt["wihT_f1"], "whhT_l1": wt["whhT_f1"],
                     "bias_l1": wt["bias_f1"],
                     "wihT_l2": wt["wihT_f2"], "whhT_l2": wt["whhT_f2"],
                     "bias_l2": wt["bias_f2"]}
                iA_c, iB_c = cls_idx(0, False)
            else:           # backward core
                xrev = np.take_along_axis(xs, rev, axis=1)
                m = {"xg_idx": chunked(xrev),
                     "iA2": chunked(base_rev.astype(np.int32)),
                     "iB2": chunked((ntok + base).astype(np.int32)),
                     "wihT_l1": wt["wihT_b1"], "whhT_l1": wt["whhT_b1"],
                     "bias_l1": wt["bias_b1"],
                     "wihT_l2": wt["wihT_b2"], "whhT_l2": wt["whhT_b2"],
                     "bias_l2": wt["bias_b2"]}
                iA_c, iB_c = cls_idx(Bl // 2, False)
            m["icA"], m["icB"] = iA_c, iB_c
            m.update(m_common)
            in_maps[core] = m
    return in_maps


_CACHED = {}


def kernel(**inputs) -> np.ndarray:
    if "nc" not in _CACHED:
        nc = bacc.Bacc("TRN2", target_bir_lowering=False, debug=False,
                       num_devices=NC)
        _build(nc)
        nc.compile()
        _CACHED["nc"] = nc
    nc = _CACHED["nc"]
    in_maps = _prep_inputs(inputs)
    res = run_bass_kernel_spmd(nc, in_maps, core_ids=list(range(NC)), trace=False)
    out = np.empty((B, T, TAGS), np.float32)
    for p in range(NPAIR):
        for half, core in ((0, p), (1, p + NPAIR)):
            lt = res.results[core]["logitsT"]          # [TAGS, ntok//2]
            seqs = lt.T.reshape(BL // 2, T, TAGS)
            b0 = BL * p + half * (BL // 2)
            out[b0:b0 + BL // 2] = seqs
    return out.astype(np.float32)


# revision 21
# speedup vs baseline: 1.0107x; 1.0107x over previous
"""BiLSTM tagger kernel, direction-parallel sharding over 8 NeuronCores.

Core c in 0..3 runs the FORWARD direction for sequences [32c, 32c+32);
core c+4 runs the BACKWARD direction for the same sequences (inputs
pre-reversed per sequence length on the host). Every matmul therefore has
M=32 batch rows instead of 16 — recurrent matmul cost is N-bound, so this
halves per-core PE work per step — and each core runs ONE scan per layer,
halving ScalarE/VectorE contention.

Between layers, the forward/backward halves are exchanged with a pairwise
AllGather (replica groups {c, c+4}) of the layer's scan-order output into
hpair [2*ntok, H]: slot 0 = lower rank = forward data on BOTH cores, so
the program is identical across cores; all per-core asymmetry (reversal
indices, which weights, which token half of the classifier) lives in
host-computed input tensors. Gate order i,f,o,g (sigmoid block first),
all matmuls bf16, PSUM fp32, gx injected into PSUM via identity matmuls
one step ahead.
"""

import sys

for _p in ("/opt/trn_rl_repo",):
    if _p not in sys.path:
        sys.path.append(_p)

import numpy as np
import ml_dtypes

import concourse.bass as bass
import concourse.tile as tile
from concourse import bacc, mybir
from concourse.bass import IndirectOffsetOnAxis
from concourse.bass_utils import run_bass_kernel_spmd

F32 = mybir.dt.float32
BF16 = mybir.dt.bfloat16
I32 = mybir.dt.int32
AF = mybir.ActivationFunctionType
ALU = mybir.AluOpType

B, T, V, E, H, TAGS = 128, 512, 50000, 256, 512, 64
NC = 8
NPAIR = NC // 2          # 4 sequence groups
BL = B // NPAIR          # 32 sequences per core (one direction each)
G = 4 * H
GROUPS = [[c, c + NPAIR] for c in range(NPAIR)]

# gate order i,g,f,o: half A = (i,g) finishes first so t2 = sig(i)*tanh(g)
# overlaps half B's matmuls; half B = (f,o) needs only ONE sigmoid call
_GATE_PERM = np.concatenate([
    np.arange(0, H), np.arange(2 * H, 3 * H), np.arange(H, 2 * H),
    np.arange(3 * H, 4 * H)])


def _build(nc, Tn=T, Bl=BL, TC=2, RC=4):
    ntok = Bl * Tn            # 16384 per core
    nchunk = ntok // 128      # 128
    ncls = ntok // 2 // 128   # 64 classifier chunks (half the pair's tokens)
    KE = E // 128
    KH2 = 2 * H // 128
    KH = H // 128

    # ---- dram I/O (per-core data resolves fwd/bwd asymmetry) ----
    emb = nc.dram_tensor("emb", [V, E], F32, kind="ExternalInput")
    xg_idx = nc.dram_tensor("xg_idx", [128, nchunk], I32, kind="ExternalInput")
    iA2 = nc.dram_tensor("iA2", [128, nchunk], I32, kind="ExternalInput")
    iB2 = nc.dram_tensor("iB2", [128, nchunk], I32, kind="ExternalInput")
    icA = nc.dram_tensor("icA", [128, ncls], I32, kind="ExternalInput")
    icB = nc.dram_tensor("icB", [128, ncls], I32, kind="ExternalInput")
    mask = nc.dram_tensor("mask", [Bl, Tn], F32, kind="ExternalInput")
    ident = nc.dram_tensor("ident", [32, 32], BF16, kind="ExternalInput")

    wih, whh, biasd = {}, {}, {}
    for l, din in (("l1", E), ("l2", 2 * H)):
        wih[l] = nc.dram_tensor(f"wihT_{l}", [din, G], BF16, kind="ExternalInput")
        whh[l] = nc.dram_tensor(f"whhT_{l}", [H, G], BF16, kind="ExternalInput")
        biasd[l] = nc.dram_tensor(f"bias_{l}", [128, G], F32, kind="ExternalInput")
    wcls = nc.dram_tensor("wclsT", [2 * H, TAGS], BF16, kind="ExternalInput")
    bcls = nc.dram_tensor("bcls", [TAGS, 1], F32, kind="ExternalInput")

    gx = {l: nc.dram_tensor(f"gx_{l}", [ntok, G], BF16) for l in ("l1", "l2")}
    hloc = {l: nc.dram_tensor(f"hloc_{l}", [ntok, H], BF16) for l in ("l1", "l2")}
    hpair = {l: nc.dram_tensor(f"hpair_{l}", [2 * ntok, H], BF16)
             for l in ("l1", "l2")}
    logitsT = nc.dram_tensor("logitsT", [TAGS, ntok // 2], F32,
                             kind="ExternalOutput")

    with tile.TileContext(nc) as tc:
        with tc.tile_pool(name="const", bufs=1) as cpool:
            def load_const(nm, shape, dt, src_ap):
                t = cpool.tile(shape, dt, name=nm, tag=nm)
                nc.gpsimd.dma_start(t[:], src_ap)
                return t

            xg_sb = load_const("xg_sb", [128, nchunk], I32, xg_idx[:])
            iA2_sb = load_const("iA2_sb", [128, nchunk], I32, iA2[:])
            iB2_sb = load_const("iB2_sb", [128, nchunk], I32, iB2[:])
            icA_sb = load_const("icA_sb", [128, ncls], I32, icA[:])
            icB_sb = load_const("icB_sb", [128, ncls], I32, icB[:])
            mask_sb = load_const("mask_sb", [Bl, Tn], F32, mask[:])
            id_sb = load_const("id_sb", [32, 32], BF16, ident[:])
            bcls_sb = load_const("bcls_sb", [TAGS, 1], F32, bcls[:])
            bias_sb = {l: load_const(f"bias_sb_{l}", [128, G], F32, biasd[l][:])
                       for l in ("l1", "l2")}
            wcls_sb = cpool.tile([128, KH2, TAGS], BF16, name="wcls_sb")
            for k in range(KH2):
                nc.gpsimd.dma_start(wcls_sb[:, k, :], wcls[128 * k:128 * (k + 1), :])

            # layer-1 proj (emb gather) + scan + exchange
            _proj(nc, tc, nchunk, KE, wih["l1"], bias_sb["l1"], gx["l1"],
                  emb, xg_sb, None, None, is_emb=True)
            _scan(nc, tc, Tn, Bl, TC, RC, KH, whh["l1"], gx["l1"], hloc["l1"],
                  mask_sb, id_sb)
            nc.gpsimd.collective_compute(
                "AllGather", ALU.bypass, GROUPS,
                ins=[hloc["l1"][:]], outs=[hpair["l1"][:]])
            # layer-2 proj (gathers from hpair) + scan + exchange
            _proj(nc, tc, nchunk, KH2, wih["l2"], bias_sb["l2"], gx["l2"],
                  hpair["l1"], None, iA2_sb, iB2_sb, is_emb=False)
            _scan(nc, tc, Tn, Bl, TC, RC, KH, whh["l2"], gx["l2"], hloc["l2"],
                  mask_sb, id_sb)
            nc.gpsimd.collective_compute(
                "AllGather", ALU.bypass, GROUPS,
                ins=[hloc["l2"][:]], outs=[hpair["l2"][:]])

            # classifier over this core's half of the pair's tokens
            with tc.tile_pool(name="cls", bufs=3) as gp, \
                 tc.tile_pool(name="clsT", bufs=3) as gtp, \
                 tc.tile_pool(name="clsps", bufs=4, space="PSUM") as pp, \
                 tc.tile_pool(name="clso", bufs=3) as op:
                for c in range(ncls):
                    o2 = gp.tile([128, 2 * H], BF16, tag="in")
                    nc.gpsimd.indirect_dma_start(
                        out=o2[:, 0:H], out_offset=None, in_=hpair["l2"][:],
                        in_offset=IndirectOffsetOnAxis(ap=icA_sb[:, c:c + 1], axis=0))
                    nc.gpsimd.indirect_dma_start(
                        out=o2[:, H:2 * H], out_offset=None, in_=hpair["l2"][:],
                        in_offset=IndirectOffsetOnAxis(ap=icB_sb[:, c:c + 1], axis=0))
                    o2T = gtp.tile([128, KH2, 128], BF16, tag="inT")
                    for k in range(KH2):
                        nc.sync.dma_start_transpose(
                            o2T[:, k, :], o2[:, 128 * k:128 * (k + 1)])
                    ps = pp.tile([TAGS, 128], F32, name="clsps_t")
                    for k in range(KH2):
                        nc.tensor.matmul(ps[:], wcls_sb[:, k, :], o2T[:, k, :],
                                         start=(k == 0), stop=(k == KH2 - 1))
                    lg = op.tile([TAGS, 128], F32, tag="lg")
                    nc.scalar.activation(lg[:], ps[:], AF.Identity,
                                         bias=bcls_sb[:, 0:1])
                    nc.gpsimd.dma_start(logitsT[:, 128 * c:128 * (c + 1)], lg[:])
    return nc


def _proj(nc, tc, nchunk, KD, wih_d, bias_t, gx_d, src, emb_idx, iA, iB, is_emb):
    """gx = input @ W_ih^T + b in scan-time order. Layer 1: fp32 emb row
    gather + cast. Layer 2: two bf16 row gathers from hpair."""
    D = KD * 128
    G_ = G
    with tc.tile_pool(name="pw", bufs=1) as wpool, \
         tc.tile_pool(name="pg", bufs=3) as gpool, \
         tc.tile_pool(name="pgT", bufs=3) as tpool, \
         tc.tile_pool(name="pps", bufs=4, space="PSUM") as ppool, \
         tc.tile_pool(name="pout", bufs=3) as opool:
        wsb = wpool.tile([128, KD, G_], BF16, tag="w", name="wih_sb")
        for k in range(KD):
            nc.gpsimd.dma_start(wsb[:, k, :], wih_d[128 * k:128 * (k + 1), :])
        for c in range(nchunk):
            if is_emb:
                e32 = gpool.tile([128, D], F32, tag="e32")
                nc.gpsimd.indirect_dma_start(
                    out=e32[:], out_offset=None, in_=src[:],
                    in_offset=IndirectOffsetOnAxis(ap=emb_idx[:, c:c + 1], axis=0))
                xin = gpool.tile([128, D], BF16, tag="e16")
                nc.vector.tensor_copy(xin[:], e32[:])
            else:
                xin = gpool.tile([128, D], BF16, tag="e16")
                nc.gpsimd.indirect_dma_start(
                    out=xin[:, 0:H], out_offset=None, in_=src[:],
                    in_offset=IndirectOffsetOnAxis(ap=iA[:, c:c + 1], axis=0))
                nc.gpsimd.indirect_dma_start(
                    out=xin[:, H:2 * H], out_offset=None, in_=src[:],
                    in_offset=IndirectOffsetOnAxis(ap=iB[:, c:c + 1], axis=0))
            xT = tpool.tile([128, KD, 128], BF16, tag="xT")
            for k in range(KD):
                nc.sync.dma_start_transpose(xT[:, k, :], xin[:, 128 * k:128 * (k + 1)])
            gout = opool.tile([128, G_], BF16, tag="gout")
            for n in range(G_ // 512):
                ps = ppool.tile([128, 512], F32, name="pps")
                for k in range(KD):
                    nc.tensor.matmul(
                        ps[:], xT[:, k, :], wsb[:, k, 512 * n:512 * (n + 1)],
                        start=(k == 0), stop=(k == KD - 1))
                nc.vector.tensor_tensor(
                    out=gout[:, 512 * n:512 * (n + 1)], in0=ps[:],
                    in1=bias_t[:, 512 * n:512 * (n + 1)], op=ALU.add)
            nc.gpsimd.dma_start(gx_d[128 * c:128 * (c + 1), :], gout[:])


def _scan(nc, tc, Tn, Bl, TC, RC, KH, whh_d, gx_d, hout_d, mask_sb, id_sb):
    """Single-direction scan, M=32 batch. Same software-pipelined gx
    injection as the 2-scan variant, one scan per core."""
    gxv = gx_d.ap().rearrange("(b t) d -> b t d", b=Bl)
    houtv = hout_d.ap().rearrange("(b t) d -> b t d", b=Bl)
    H2 = 2 * H
    with tc.tile_pool(name="sw", bufs=1) as wpool, \
         tc.tile_pool(name="sgx", bufs=4) as gxpool, \
         tc.tile_pool(name="sst", bufs=1) as stpool, \
         tc.tile_pool(name="sps", bufs=4, space="PSUM") as pspool, \
         tc.tile_pool(name="swk", bufs=3) as wkpool, \
         tc.tile_pool(name="shT", bufs=3) as htpool, \
         tc.tile_pool(name="srng", bufs=3) as rpool:
        wsb = wpool.tile([128, KH, G], BF16, tag="whh", name="whh_sb")
        for k in range(KH):
            nc.gpsimd.dma_start(wsb[:, k, :], whh_d[128 * k:128 * (k + 1), :])
        hT = [htpool.tile([128, KH * Bl], BF16, tag="hT", name="hT0")]
        nc.vector.memset(hT[0][:], 0.0)
        c_st = stpool.tile([Bl, H], F32, tag="c", name="c_st")
        nc.vector.memset(c_st[:], 0.0)
        gxc = {}
        gA = [None]
        gB = [None]
        ring = [None]
        nwin = (Tn + TC - 1) // TC

        def load_gx(w):
            tl = gxpool.tile([Bl, TC, G], BF16, tag="gx", name="gxc")
            nc.gpsimd.dma_start(tl[:], gxv[:, w * TC:(w + 1) * TC, :])
            gxc[w] = tl
            gxc.pop(w - 3, None)

        def inject(tt):
            gA[0] = pspool.tile([Bl, H2], F32, tag="ps", name="gA")
            gB[0] = pspool.tile([Bl, H2], F32, tag="ps", name="gB")
            gxt = gxc[tt // TC]
            for half, lo in ((gA[0], 0), (gB[0], H2)):
                for n in range(2):
                    nc.tensor.matmul(
                        half[:, 512 * n:512 * (n + 1)], id_sb[:],
                        gxt[:, tt % TC, lo + 512 * n:lo + 512 * (n + 1)],
                        start=True, stop=False, skip_group_check=True)

        load_gx(0)
        if nwin > 1:
            load_gx(1)
        inject(0)
        for t in range(Tn):
            gAc, gBc = gA[0], gB[0]
            gact = wkpool.tile([Bl, G], F32, tag="gact", name="gact")
            t1 = wkpool.tile([Bl, H], F32, tag="t1", name="t1")
            t2 = wkpool.tile([Bl, H], F32, tag="t2", name="t2")
            if t % RC == 0:
                ring[0] = rpool.tile([Bl, RC, H], BF16, tag="ring", name="ring")
            for half, cols in ((gAc, (0, 1)), (gBc, (2, 3))):
                for n in cols:
                    dst_lo = 512 * (n % 2)
                    for k in range(KH):
                        nc.tensor.matmul(
                            half[:, dst_lo:dst_lo + 512],
                            hT[0][:, Bl * k:Bl * (k + 1)],
                            wsb[:, k, 512 * n:512 * (n + 1)],
                            start=False, stop=(k == KH - 1),
                            skip_group_check=True)
                if half is gAc:
                    # i,g activations + t2 run under half B's matmuls
                    nc.scalar.activation(gact[:, 0:H], gAc[:, 0:H], AF.Sigmoid)
                    nc.scalar.activation(gact[:, H:H2], gAc[:, H:H2], AF.Tanh)
                    nc.vector.tensor_tensor(out=t2[:], in0=gact[:, 0:H],
                                            in1=gact[:, H:H2], op=ALU.mult)
            # sig(f) depends only on half B's f-columns (n=2 chain), so it
            # fires ~0.9us before the o-columns finish; sig(o) is off the
            # c-path and runs while the DVE does t1/c
            nc.scalar.activation(gact[:, H2:3 * H], gBc[:, 0:H], AF.Sigmoid)
            nc.vector.tensor_tensor(out=t1[:], in0=gact[:, H2:3 * H],
                                    in1=c_st[:], op=ALU.mult)
            nc.vector.tensor_tensor(out=c_st[:], in0=t1[:], in1=t2[:], op=ALU.add)
            tch = wkpool.tile([Bl, H], F32, tag="tch", name="tch")
            # sig(o) before tanh(c) in the ScalarE FIFO: its input (the
            # o-columns' matmul chain) is ready before c is
            nc.scalar.activation(gact[:, 3 * H:G], gBc[:, H:H2], AF.Sigmoid)
            nc.scalar.activation(tch[:], c_st[:], AF.Tanh)
            h16 = wkpool.tile([Bl, H], BF16, tag="h16", name="h16")
            nc.vector.tensor_tensor(out=h16[:], in0=gact[:, 3 * H:G],
                                    in1=tch[:], op=ALU.mult)
            hT_ps = pspool.tile([128, KH * Bl], F32, tag="ps", name="hT_ps")
            if t + 1 < Tn:
                if (t + 1) % TC == 0 and (t + 1) // TC + 1 < nwin:
                    load_gx((t + 1) // TC + 1)
                inject(t + 1)
            # per-chunk copy right behind each transpose so next step's
            # first matmul (which reads only chunk 0) starts early
            hTn = htpool.tile([128, KH * Bl], BF16, tag="hT", name="hTn")
            for k in range(KH):
                nc.tensor.matmul(hT_ps[:, Bl * k:Bl * (k + 1)],
                                 h16[:, 128 * k:128 * (k + 1)], id_sb[:],
                                 start=True, stop=True)
                nc.scalar.activation(hTn[:, Bl * k:Bl * (k + 1)],
                                     hT_ps[:, Bl * k:Bl * (k + 1)], AF.Copy)
            hT[0] = hTn
            nc.vector.tensor_scalar_mul(ring[0][:, t % RC, :], h16[:],
                                        mask_sb[:, t:t + 1])
            if (t + 1) % RC == 0:
                t0r = t + 1 - RC
                nc.gpsimd.dma_start(houtv[:, t0r:t0r + RC, :], ring[0][:, :, :])


def _prep_inputs(inputs, Tn=T, Bl=BL):
    x = np.asarray(inputs["x"]).astype(np.int32)
    lengths = np.asarray(inputs["lengths"]).astype(np.int32)
    emb = np.asarray(inputs["emb"], dtype=np.float32)
    ntok = Bl * Tn
    bf = ml_dtypes.bfloat16

    wt = {}
    for s in ("f1", "b1", "f2", "b2"):
        w_ih = np.asarray(inputs[f"W_ih_{s}"], np.float32)[_GATE_PERM]
        w_hh = np.asarray(inputs[f"W_hh_{s}"], np.float32)[_GATE_PERM]
        b = np.asarray(inputs[f"b_{s}"], np.float32)[_GATE_PERM]
        wt[f"wihT_{s}"] = np.ascontiguousarray(w_ih.T).astype(bf)
        wt[f"whhT_{s}"] = np.ascontiguousarray(w_hh.T).astype(bf)
        wt[f"bias_{s}"] = np.tile(b.reshape(1, G), (128, 1))
    com = {"emb": emb, "ident": np.eye(32, dtype=bf),
           "wclsT": np.ascontiguousarray(
               np.asarray(inputs["W_cls"], np.float32).T).astype(bf),
           "bcls": np.asarray(inputs["b_cls"], np.float32).reshape(TAGS, 1)}

    def chunked(a):
        return np.ascontiguousarray(a.reshape(-1).reshape(-1, 128).T)

    in_maps = [None] * NC
    for p in range(NPAIR):
        xs = x[Bl * p:Bl * (p + 1), :Tn]
        ls = np.minimum(lengths[Bl * p:Bl * (p + 1)], Tn)
        ts = np.arange(Tn)[None, :]
        rev = np.where(ts < ls[:, None], ls[:, None] - 1 - ts, ts)  # [Bl,Tn]
        base = np.arange(Bl)[:, None] * Tn + ts                      # natural
        base_rev = np.arange(Bl)[:, None] * Tn + rev                 # reversed
        m_common = {"mask": (ts < ls[:, None]).astype(np.float32)}
        m_common.update(com)

        # classifier token halves: fwd core -> seqs [0:Bl//2), bwd -> rest
        def cls_idx(b0, slotA_rev):
            tok = (np.arange(b0 * Tn, (b0 + Bl // 2) * Tn))
            bb, tt2 = tok // Tn, tok % Tn
            iA_ = bb * Tn + tt2
            iB_ = ntok + bb * Tn + rev[bb, tt2]
            if slotA_rev:
                pass
            return chunked(iA_.astype(np.int32)), chunked(iB_.astype(np.int32))

        for half, core in ((0, p), (1, p + NPAIR)):
            if half == 0:   # forward core
                m = {"xg_idx": chunked(xs),
                     "iA2": chunked(base.astype(np.int32)),
                     "iB2": chunked((ntok + base_rev).astype(np.int32)),
                     "wihT_l1": wt["wihT_f1"], "whhT_l1": wt["whhT_f1"],
                     "bias_l1": wt["bias_f1"],
                     "wihT_l2": wt["wihT_f2"], "whhT_l2": wt["whhT_f2"],
                     "bias_l2": wt["bias_f2"]}
                iA_c, iB_c = cls_idx(0, False)
            else:           # backward core
                xrev = np.take_along_axis(xs, rev, axis=1)
                m = {"xg_idx": chunked(xrev),
                     "iA2": chunked(base_rev.astype(np.int32)),
                     "iB2": chunked((ntok + base).astype(np.int32)),
                     "wihT_l1": wt["wihT_b1"], "whhT_l1": wt["whhT_b1"],
                     "bias_l1": wt["bias_b1"],
                     "wihT_l2": wt["wihT_b2"], "whhT_l2": wt["whhT_b2"],
                     "bias_l2": wt["bias_b2"]}
                iA_c, iB_c = cls_idx(Bl // 2, False)
            m["icA"], m["icB"] = iA_c, iB_c
            m.update(m_common)
            in_maps[core] = m
    return in_maps


_CACHED = {}


def kernel(**inputs) -> np.ndarray:
    if "nc" not in _CACHED:
        nc = bacc.Bacc("TRN2", target_bir_lowering=False, debug=False,
                       num_devices=NC)
        _build(nc)
        nc.compile()
        _CACHED["nc"] = nc
    nc = _CACHED["nc"]
    in_maps = _prep_inputs(inputs)
    res = run_bass_kernel_spmd(nc, in_maps, core_ids=list(range(NC)), trace=False)
    out = np.empty((B, T, TAGS), np.float32)
    for p in range(NPAIR):
        for half, core in ((0, p), (1, p + NPAIR)):
            lt = res.results[core]["logitsT"]          # [TAGS, ntok//2]
            seqs = lt.T.reshape(BL // 2, T, TAGS)
            b0 = BL * p + half * (BL // 2)
            out[b0:b0 + BL // 2] = seqs
    return out.astype(np.float32)
